# revision 35
# baseline (speedup 1.0000x reference)
"""GQA causal attention (B=1, T=4096, D=1024, HQ=16, HKV=4, HD=64) on 8 trn2
NeuronCores via Bass/Tile.

Default mode "v4" (~294us HW): zero collectives -- every core redundantly
projects the full K^T/V from bf16 x^T one 512-token chunk at a time, and those
projection sub-groups (plus the output-projection panels) are drained one at a
time into the PE bubble between each attention group's score matmuls and its
ctx matmuls, where the PE otherwise idles waiting for the ACT-engine exp.
Modes v1/v2/v3 (AllGather-based KV distribution) are retained for reference;
the collective chain costs ~120us of wall latency (barrier rendezvous skew +
trigger delay + transfer) and loses to redundant compute.

Sharding: block-cyclic sequence-parallel. The 4096 query tokens are split into
64 blocks of 64; core i owns blocks {i, 8+i, ..., 56+i} (512 q tokens). Every
core runs the SAME program (SPMD requirement): for its j-th block it processes
k-tiles [0, 4*(j+1)) — a core-independent conservative causal extent — and a
host-supplied per-core boundary mask zeroes the non-causal tail, so per-core
work is uniform AND balanced (each core ~1/8 of the causal area).

Layout strategy (avoids all on-device transposes):
  - host passes x^T; scores are computed as S^T[k, q] = (K^T)^T-tiles @ Q^T
    with k on partitions, so the softmax denominator is obtained by appending
    a ones-column to the V stationary ([V|1]) and the exp is a pure
    elementwise ACT pass PSUM->SBUF.
  - normalization is deferred: ctx^T = (sum_k e^s V) is divided by the
    rowsum (row 64 of the [V|1] matmul output) after the k-loop, via a
    reciprocal + K=1 broadcast-matmul.
  - Wq/Wo columns/rows are host-permuted so two heads stack into 128
    partitions everywhere (even-kv heads on partitions 0-63, odd-kv on
    64-127), which also lets score matmuls (contraction dim = head_dim = 64)
    run pairwise-packed in the PE array via tile_position row groups.

dtypes: projections and the output matmul run in float32r (full-rate fp32
variant, moving dim >= 256); score/ctx matmuls run in bf16 (N=64 would put
f32r in its slow mode); softmax accumulation is fp32 in PSUM.
"""

import os
import sys

sys.path.insert(0, "/opt/trn_rl_repo")

import numpy as np
import ml_dtypes

import concourse.bass as bass
import concourse.bacc as bacc
import concourse.mybir as mybir
import concourse.tile as tile
from concourse.bass_utils import run_bass_kernel_spmd

# ---------------------------------------------------------------- constants
B, T, D = 1, 4096, 1024
HQ, HKV, HD = 16, 4, 64
G = HQ // HKV          # 4 q heads per kv head
NC = 8                 # cores
QB = 64                # q block size
NBLK = T // QB         # 64 blocks total
BPC = NBLK // NC       # 8 blocks per core
LQ = QB * BPC          # 512 local q tokens per core
DT = D // 128          # 8 contraction tiles over D
NKT = T // 128         # 32 k-tiles
F32 = mybir.dt.float32
F32R = mybir.dt.float32r
BF16 = mybir.dt.bfloat16
BF16NP = ml_dtypes.bfloat16

# head pairing: pair tile m holds (LO[m] on partitions 0-63, HI[m] on 64-127).
# LO = heads of even kv-heads, HI = heads of odd kv-heads, so score matmuls of
# a lo head (stationary K^T at partitions 0-63) can be row-group-packed with a
# hi head (partitions 64-127).
LO = [0, 1, 2, 3, 8, 9, 10, 11]
HI = [4, 5, 6, 7, 12, 13, 14, 15]


def _local_cols(i):
    """Global token indices owned by core i, in local order."""
    return np.concatenate(
        [QB * (NC * j + i) + np.arange(QB) for j in range(BPC)]
    )


def _band_mask(i):
    """[4, 128, 64] multiplicative causal mask for the last k-quartet of any
    block: valid iff 128*kt2 + p <= 64*i + f."""
    kt2 = np.arange(4)[:, None, None]
    p = np.arange(128)[None, :, None]
    f = np.arange(64)[None, None, :]
    return (128 * kt2 + p <= 64 * i + f).astype(BF16NP)


def _r(ap):
    return ap.bitcast(F32R)


# ---------------------------------------------------------------- program
RC = 3  # v3: leading 512-token chunks projected redundantly on every core


def build_nc(mode="v2"):
    nc = bacc.Bacc(None)
    if mode == "v4":
        xc_d = nc.declare_dram_parameter("xT_cyc", [D, LQ], BF16, isOutput=False)
        xf_d = nc.declare_dram_parameter("xT_full", [D, T], BF16, isOutput=False)
        wq_d = nc.declare_dram_parameter("Wq_perm", [D, HQ * HD], BF16, isOutput=False)
        wk_d = nc.declare_dram_parameter("Wk_n", [D, HKV * HD], BF16, isOutput=False)
        wv_d = nc.declare_dram_parameter("Wv_n", [D, HKV * HD], BF16, isOutput=False)
        wo_d = nc.declare_dram_parameter("Wo_perm", [HQ * HD, D], F32R, isOutput=False)
        bm_d = nc.declare_dram_parameter("bmask", [4, 128, QB], BF16, isOutput=False)
        on_d = nc.declare_dram_parameter("ones_c", [1, HD], F32R, isOutput=False)
        out_d = nc.declare_dram_parameter("out_loc", [LQ, D], F32, isOutput=True)
        with tile.TileContext(nc) as tc:
            _emit_v4(nc, tc, xc_d, xf_d, wq_d, wk_d, wv_d, wo_d, bm_d, on_d, out_d)
        nc.finalize()
        return nc
    if mode == "v3":
        xc_d = nc.declare_dram_parameter("xT_cyc", [D, LQ], BF16, isOutput=False)
        xg_d = nc.declare_dram_parameter("xT_ctg", [D, LQ], BF16, isOutput=False)
        xh_d = nc.declare_dram_parameter("x_head", [D, 512 * RC], BF16, isOutput=False)
        wq_d = nc.declare_dram_parameter("Wq_perm", [D, HQ * HD], BF16, isOutput=False)
        wk_d = nc.declare_dram_parameter("Wk_n", [D, HKV * HD], BF16, isOutput=False)
        wv_d = nc.declare_dram_parameter("Wv_n", [D, HKV * HD], BF16, isOutput=False)
        wo_d = nc.declare_dram_parameter("Wo_perm", [HQ * HD, D], F32R, isOutput=False)
        bm_d = nc.declare_dram_parameter("bmask", [4, 128, QB], BF16, isOutput=False)
        on_d = nc.declare_dram_parameter("ones_c", [1, HD], F32R, isOutput=False)
        out_d = nc.declare_dram_parameter("out_loc", [LQ, D], F32, isOutput=True)
        with tile.TileContext(nc) as tc:
            _emit_v3(nc, tc, xc_d, xg_d, xh_d, wq_d, wk_d, wv_d, wo_d, bm_d, on_d, out_d)
        nc.finalize()
        return nc
    xo_d = nc.declare_dram_parameter("xT_own", [D, LQ], F32R, isOutput=False)
    if mode == "v1":
        xf_d = nc.declare_dram_parameter("xT_full", [D, T], BF16, isOutput=False)
        wdt = BF16
    else:
        xf_d = None
        wdt = F32R
    wq_d = nc.declare_dram_parameter("Wq_perm", [D, HQ * HD], F32R, isOutput=False)
    wk_d = nc.declare_dram_parameter("Wk_n", [D, HKV * HD], wdt, isOutput=False)
    wv_d = nc.declare_dram_parameter("Wv_n", [D, HKV * HD], wdt, isOutput=False)
    wo_d = nc.declare_dram_parameter("Wo_perm", [HQ * HD, D], F32R, isOutput=False)
    bm_d = nc.declare_dram_parameter("bmask", [4, 128, QB], BF16, isOutput=False)
    on_d = nc.declare_dram_parameter("ones_c", [1, HD], F32R, isOutput=False)
    out_d = nc.declare_dram_parameter("out_loc", [LQ, D], F32, isOutput=True)

    with tile.TileContext(nc) as tc:
        _emit(nc, tc, mode, xo_d, xf_d, wq_d, wk_d, wv_d, wo_d, bm_d, on_d, out_d)
    nc.finalize()
    return nc


def _emit(nc, tc, mode, xo_d, xf_d, wq_d, wk_d, wv_d, wo_d, bm_d, on_d, out_d):
    from contextlib import ExitStack

    es = ExitStack()
    with es:
        sb = es.enter_context(tc.tile_pool(name="sb", bufs=2))
        sb3 = es.enter_context(tc.tile_pool(name="sb3", bufs=6))
        res = es.enter_context(tc.tile_pool(name="res", bufs=1))
        ps2 = es.enter_context(tc.tile_pool(name="ps2", bufs=2, space="PSUM"))

        # ---------------- resident tensors
        xo = res.tile([128, DT, LQ], F32R, tag="xo")          # x^T own cols
        nc.sync.dma_start(xo[:], xo_d.rearrange("(dt p) q -> p dt q", p=128))
        wdt = BF16 if mode == "v1" else F32R
        wk = res.tile([128, DT, HKV * HD], wdt, tag="wk")
        nc.sync.dma_start(wk[:], wk_d.rearrange("(dt p) h -> p dt h", p=128))
        wv = res.tile([128, DT, HKV * HD], wdt, tag="wv")
        nc.sync.dma_start(wv[:], wv_d.rearrange("(dt p) h -> p dt h", p=128))
        bm = res.tile([128, 4, QB], BF16, tag="bm")          # band masks
        nc.sync.dma_start(bm[:], bm_d.rearrange("k p f -> p k f"))

        kt_sb = [res.tile([128, T], BF16, tag=f"kt{h2}", name=f"kt{h2}") for h2 in range(2)]
        v_sb = res.tile([128, NKT, HKV, HD + 1], BF16, tag="v")  # [V | 1]
        # Q^T grouped by kv-pair: qg_sb[h2][64*hs:, g, :] = q-head 4*(2*h2+hs)+g
        qg_sb = [
            res.tile([128, G, LQ], BF16, tag=f"qg{h2}", name=f"qg{h2}")
            for h2 in range(2)
        ]
        ctx_sb = res.tile([128, 8, LQ], F32R, tag="ctx")      # normalized ctx^T
        ones_sb = res.tile([1, HD], F32R, tag="ones")
        nc.sync.dma_start(ones_sb[:], on_d[:])
        nc.vector.memset(v_sb[:, :, :, HD : HD + 1], 1.0)

        # ---------------- P1a: Q^T projection (f32r), scaled by HD^-0.5
        # Wq lives in the "wbig" slot; Wo reuses the same slot later (the
        # phases are sequential, the pool dependency-orders the reuse).
        wqt = sb.tile([128, DT, HQ * HD], F32R, tag="wbig", name="wqt")
        nc.sync.dma_start(wqt[:], wq_d.rearrange("(dt p) h -> p dt h", p=128))
        for m in range(8):
            psq = ps2.tile([128, LQ], F32, tag="pacc", name=f"psq{m}")
            for d in range(DT):
                nc.tensor.matmul(
                    psq[:],
                    wqt[:, d, 128 * m : 128 * (m + 1)],
                    xo[:, d, :],
                    start=(d == 0),
                    stop=(d == DT - 1),
                )
            # cast to bf16 with the 1/sqrt(HD) score scale folded in
            nc.vector.tensor_scalar_mul(
                qg_sb[m // 4][:, m % 4, :], psq[:], float(HD) ** -0.5
            )


        if mode == "v1":
            # ------------ P1b/c: K^T and V projections from bf16 x^T (full T)
            for c in range(8):  # 512-token chunks
                xf = sb.tile([128, DT, 512], BF16, tag="xf")
                nc.sync.dma_start(
                    xf[:],
                    xf_d.rearrange("(dt p) t -> p dt t", p=128)[:, :, 512 * c : 512 * (c + 1)],
                )
                for h2 in range(2):  # K^T: kv-pair tiles (kv0|kv1), (kv2|kv3)
                    psk = ps2.tile([128, 512], F32, tag="scores", name="psk")
                    for d in range(DT):
                        nc.tensor.matmul(
                            psk[:],
                            wk[:, d, 128 * h2 : 128 * (h2 + 1)],
                            xf[:, d, :],
                            start=(d == 0),
                            stop=(d == DT - 1),
                        )
                    nc.vector.tensor_copy(kt_sb[h2][:, 512 * c : 512 * (c + 1)], psk[:])
                for tq in range(4):  # V natural [t, d] via x^T-stationary matmuls
                    kt = 4 * c + tq
                    psv = ps2.tile([128, HKV * HD], F32, tag="scores", name="psv")
                    for d in range(DT):
                        nc.tensor.matmul(
                            psv[:],
                            xf[:, d, 128 * tq : 128 * (tq + 1)],
                            wv[:, d, :],
                            start=(d == 0),
                            stop=(d == DT - 1),
                        )
                    nc.vector.tensor_copy(
                        v_sb[:, kt, :, 0:HD],
                        psv.rearrange("p (h e) -> p h e", h=HKV),
                    )

        else:
            _emit_kv_allgather(nc, tc, res, sb, ps2, xo, wk, wv, kt_sb, v_sb)

        # ---------------- P2: attention over blocks
        for j in range(BPC):
            nkp = 2 * (j + 1)  # k-tile pairs this block
            ctx_ps = [
                ps2.tile([HD + 1, 8 * QB], F32, tag="ctx", name=f"ctxps{h2}_{j}")
                for h2 in range(2)
            ]
            for kp in range(nkp):
                for h2 in range(2):
                    s_ps = ps2.tile([128, 2, 2, G, QB], F32, tag="scores")
                    qsl = slice(QB * j, QB * (j + 1))
                    for kt2 in range(2):
                        kt = 2 * kp + kt2
                        ksl = slice(128 * kt, 128 * (kt + 1))
                        for hs in range(2):
                            # one matmul covers all G q-heads of this kv head
                            nc.tensor.matmul(
                                s_ps[:, hs, kt2, :, :],
                                kt_sb[h2][64 * hs : 64 * hs + 64, ksl],
                                qg_sb[h2][64 * hs : 64 * hs + 64, :, qsl],
                                start=True, stop=True,
                                tile_position=(64 * hs, 0),
                            )
                    pt = sb3.tile([128, 2, 2, G, QB], BF16, tag="pt")
                    nc.scalar.activation(
                        pt[:], s_ps[:], mybir.ActivationFunctionType.Exp
                    )
                    if kp >= 2 * j:  # boundary quartet: apply causal mask
                        par = kp - 2 * j
                        msk = bm[:, 2 * par : 2 * par + 2, None, :].to_broadcast(
                            (128, 2, G, QB)
                        )
                        for hs in range(2):
                            nc.vector.tensor_mul(pt[:, hs], pt[:, hs], msk)
                    for kt2 in range(2):
                        kt = 2 * kp + kt2
                        for hs in range(2):
                            kv = 2 * h2 + hs
                            # start=True only on the very first matmul into this
                            # psum tile: start marks the whole 2KB zero-region
                            # pending-zero, so each slot's first write overwrites
                            # (correct) and later writes accumulate. A second
                            # start=True would re-mark the bank and wipe other
                            # slots' partials.
                            nc.tensor.matmul(
                                ctx_ps[h2][:, 256 * hs : 256 * (hs + 1)],
                                v_sb[:, kt, kv, :],
                                pt[:, hs, kt2, :, :],
                                start=(kp == 0 and kt2 == 0 and hs == 0),
                                stop=(kp == nkp - 1 and kt2 == 1),
                                skip_group_check=True,
                            )
            # ---- normalize: ctx / rowsum (row HD of ctx_ps)
            rs = sb.tile([1, 2, 8 * QB], F32R, tag="rs")
            for h2 in range(2):
                nc.vector.tensor_copy(rs[0:1, h2, :], ctx_ps[h2][HD : HD + 1, :])
            hi_st = sb.tile([64, 8, QB], F32R, tag="hist")
            for h2 in range(2):
                # broadcast rowsum over 64 partitions FIRST, then take the
                # reciprocal on 64 lanes (a [1, N] reciprocal runs on one lane
                # at ~6.5us; this form is ~0.5us)
                bc = ps2.tile([HD, 8 * QB], F32, tag="pacc", name="bc")
                nc.tensor.matmul(
                    bc[:], ones_sb[:], rs[0:1, h2, :],
                    start=True, stop=True,
                )
                bcs = sb.tile([HD, 8 * QB], F32, tag="bcs")
                # ~51-ULP approx is plenty for a softmax denominator and ~5x
                # faster than the exact DVE reciprocal (6 cyc/elem)
                nc.vector.reciprocal_approx_fast(out=bcs[:], in_=bc[:])
                for hs in range(2):
                    for mq in range(4):
                        s = 4 * hs + mq
                        m = 4 * h2 + mq
                        ssl = slice(QB * s, QB * (s + 1))
                        if hs == 0:
                            nc.vector.tensor_mul(
                                ctx_sb[0:64, m, QB * j : QB * (j + 1)],
                                ctx_ps[h2][0:HD, ssl],
                                bcs[:, ssl],
                            )
                        else:
                            nc.vector.tensor_mul(
                                hi_st[:, m, :], ctx_ps[h2][0:HD, ssl], bcs[:, ssl]
                            )
            # partition-shift the odd-kv heads to partitions 64-127 (DMA)
            nc.sync.dma_start(
                ctx_sb[64:128, :, QB * j : QB * (j + 1)], hi_st[:]
            )

        # ---------------- P3: out = ctx @ Wo  (f32r)
        out_sb = res.tile([128, 4, D], F32, tag="osb")
        wot = sb.tile([128, 8, D], F32R, tag="wbig", name="wot")
        nc.sync.dma_start(wot[:], wo_d.rearrange("(m p) dcol -> p m dcol", p=128))
        for tt in range(4):
            for dc in range(2):
                pso = ps2.tile([128, 512], F32, tag="pacc", name=f"pso{tt}_{dc}")
                for m in range(8):
                    nc.tensor.matmul(
                        pso[:],
                        ctx_sb[:, m, 128 * tt : 128 * (tt + 1)],
                        wot[:, m, 512 * dc : 512 * (dc + 1)],
                        start=(m == 0),
                        stop=(m == 7),
                    )
                nc.vector.tensor_copy(
                    out_sb[:, tt, 512 * dc : 512 * (dc + 1)], pso[:]
                )
        nc.sync.dma_start(
            out_d.rearrange("(tt p) dcol -> p tt dcol", p=128), out_sb[:]
        )





def _emit_v3(nc, tc, xc_d, xg_d, xh_d, wq_d, wk_d, wv_d, wo_d, bm_d, on_d, out_d):
    """v3: contiguous KV-projection ownership (gather output lands in global
    token order -> few, large scatter DMAs), gathers triggered first, Q
    projection + redundant projection of the first RC chunks under the gather
    latency, and the output projection interleaved per 128-token tile."""
    from contextlib import ExitStack

    es = ExitStack()
    with es:
        sb = es.enter_context(tc.tile_pool(name="sb", bufs=2))
        sb3 = es.enter_context(tc.tile_pool(name="sb3", bufs=6))
        res = es.enter_context(tc.tile_pool(name="res", bufs=1))
        pss = es.enter_context(tc.tile_pool(name="pss", bufs=2, space="PSUM"))
        psc = es.enter_context(tc.tile_pool(name="psc", bufs=4, space="PSUM"))
        dram = es.enter_context(tc.tile_pool(name="dramkv", bufs=1, space="DRAM"))

        # ---------------- resident tensors, in DMA-arrival order of need
        xg = res.tile([128, DT, LQ], BF16, tag="xg")           # x^T contig cols
        nc.sync.dma_start(xg[:], xg_d.rearrange("(dt p) q -> p dt q", p=128))
        wk = res.tile([128, DT, HKV * HD], BF16, tag="wk")
        nc.sync.dma_start(wk[:], wk_d.rearrange("(dt p) h -> p dt h", p=128))
        wv = res.tile([128, DT, HKV * HD], BF16, tag="wv")
        nc.sync.dma_start(wv[:], wv_d.rearrange("(dt p) h -> p dt h", p=128))
        xc = res.tile([128, DT, LQ], BF16, tag="xc")           # x^T cyclic cols
        nc.sync.dma_start(xc[:], xc_d.rearrange("(dt p) q -> p dt q", p=128))
        wqt = res.tile([128, DT, HQ * HD], BF16, tag="wqt")
        nc.sync.dma_start(wqt[:], wq_d.rearrange("(dt p) h -> p dt h", p=128))
        xh = res.tile([128, DT, 512 * RC], BF16, tag="xh")     # head chunks
        nc.sync.dma_start(xh[:], xh_d.rearrange("(dt p) t -> p dt t", p=128))
        bm = res.tile([128, 4, QB], BF16, tag="bm")
        nc.sync.dma_start(bm[:], bm_d.rearrange("k p f -> p k f"))
        ones_sb = res.tile([1, HD], F32R, tag="ones")
        nc.sync.dma_start(ones_sb[:], on_d[:])
        wot = res.tile([128, 8, D], F32R, tag="wot")
        nc.sync.dma_start(wot[:], wo_d.rearrange("(m p) dcol -> p m dcol", p=128))

        kt_sb = [res.tile([128, T], BF16, tag=f"kt{h2}", name=f"kt{h2}") for h2 in range(2)]
        v_sb = res.tile([128, NKT, HKV, HD + 1], BF16, tag="v")  # [V | 1]
        qg_sb = [
            res.tile([128, G, LQ], BF16, tag=f"qg{h2}", name=f"qg{h2}")
            for h2 in range(2)
        ]
        ctx_sb = res.tile([128, 8, LQ], F32R, tag="ctx")
        out_sb = res.tile([128, 4, D], F32, tag="osb")
        ko_sb = res.tile([128, 2, LQ], BF16, tag="ko_sb")
        vo_sb = res.tile([128, 4, HKV * HD], BF16, tag="vo_sb")
        nc.vector.memset(v_sb[:, :, :, HD : HD + 1], 1.0)

        # ---------------- P0: project OWN contiguous K^T/V chunk, gather
        for h2 in range(2):
            ps = pss.tile([128, 1024], F32, tag="scores", name=f"psk{h2}")
            psk = ps[:, 0:LQ]
            for d in range(DT):
                nc.tensor.matmul(
                    psk, wk[:, d, 128 * h2 : 128 * (h2 + 1)], xg[:, d, :],
                    start=(d == 0), stop=(d == DT - 1),
                )
            nc.vector.tensor_copy(ko_sb[:, h2, :], psk)
        for tq in range(4):
            ps = pss.tile([128, 1024], F32, tag="scores", name=f"psv{tq}")
            psv = ps[:, 0 : HKV * HD]
            for d in range(DT):
                nc.tensor.matmul(
                    psv, xg[:, d, 128 * tq : 128 * (tq + 1)], wv[:, d, :],
                    start=(d == 0), stop=(d == DT - 1),
                )
            nc.vector.tensor_copy(vo_sb[:, tq, :], psv)
        ko_d = dram.tile([2 * 128, LQ], BF16, name="ko_d")
        vo_d = dram.tile([LQ, HKV * HD], BF16, name="vo_d")
        nc.sync.dma_start(ko_d.rearrange("(h2 p) q -> p h2 q", p=128), ko_sb[:])
        nc.sync.dma_start(vo_d.rearrange("(tq p) h -> p tq h", p=128), vo_sb[:])
        kg_d = dram.tile([NC * 2 * 128, LQ], BF16, name="kg_d", addr_space="Shared")
        vg_d = dram.tile([NC * LQ, HKV * HD], BF16, name="vg_d", addr_space="Shared")
        nc.gpsimd.collective_compute(
            "AllGather", mybir.AluOpType.bypass,
            replica_groups=[list(range(NC))],
            ins=[ko_d[:]], outs=[kg_d[:]],
        )
        nc.gpsimd.collective_compute(
            "AllGather", mybir.AluOpType.bypass,
            replica_groups=[list(range(NC))],
            ins=[vo_d[:]], outs=[vg_d[:]],
        )

        # ---------------- P1a: Q^T projection (bf16; HD^-0.5 folded on host)
        for m in range(8):
            ps = pss.tile([128, 1024], F32, tag="scores", name=f"psq{m}")
            psq = ps[:, 0:LQ]
            for d in range(DT):
                nc.tensor.matmul(
                    psq, wqt[:, d, 128 * m : 128 * (m + 1)], xc[:, d, :],
                    start=(d == 0), stop=(d == DT - 1),
                )
            nc.vector.tensor_copy(qg_sb[m // 4][:, m % 4, :], psq)

        # ---------------- P1b: redundant K^T/V for head chunks 0..RC-1
        for c in range(RC):
            csl = slice(512 * c, 512 * (c + 1))
            for h2 in range(2):
                ps = pss.tile([128, 1024], F32, tag="scores", name=f"hk{c}{h2}")
                psk = ps[:, 0:512]
                for d in range(DT):
                    nc.tensor.matmul(
                        psk, wk[:, d, 128 * h2 : 128 * (h2 + 1)], xh[:, d, csl],
                        start=(d == 0), stop=(d == DT - 1),
                    )
                nc.vector.tensor_copy(kt_sb[h2][:, csl], psk)
            for tq in range(4):
                ps = pss.tile([128, 1024], F32, tag="scores", name=f"hv{c}{tq}")
                psv = ps[:, 0 : HKV * HD]
                for d in range(DT):
                    nc.tensor.matmul(
                        psv,
                        xh[:, d, 512 * c + 128 * tq : 512 * c + 128 * (tq + 1)],
                        wv[:, d, :],
                        start=(d == 0), stop=(d == DT - 1),
                    )
                nc.vector.tensor_copy(
                    v_sb[:, 4 * c + tq, :, 0:HD],
                    psv.rearrange("p (h e) -> p h e", h=HKV),
                )

        # ---------------- P1c: scatter gathered chunks RC..7 (global order)
        kgv = kg_d.rearrange("(c h2 p) q -> c h2 p q", c=NC, h2=2, p=128)
        vgv = vg_d.rearrange("(c tq p) (kv e) -> c p tq kv e", c=NC, tq=4, p=128, kv=HKV)
        for c in range(RC, NC):
            for h2 in range(2):
                nc.gpsimd.dma_start(
                    kt_sb[h2][:, 512 * c : 512 * (c + 1)], kgv[c, h2]
                )
            for tq in range(4):
                nc.gpsimd.dma_start(
                    v_sb[:, 4 * c + tq, :, 0:HD], vgv[c, :, tq]
                )

        # ---------------- P2: attention blocks + interleaved P3
        for j in range(BPC):
            nkp = 2 * (j + 1)
            ctx_ps = [
                psc.tile([HD + 1, 8 * QB], F32, tag="ctx", name=f"ctxps{h2}_{j}")
                for h2 in range(2)
            ]
            for kp in range(nkp):
                for h2 in range(2):
                    s_t = pss.tile([128, 1024], F32, tag="scores", name="s_t")
                    s_ps = s_t.rearrange(
                        "p (hs kt2 g f) -> p hs kt2 g f", hs=2, kt2=2, g=G
                    )
                    qsl = slice(QB * j, QB * (j + 1))
                    for kt2 in range(2):
                        kt = 2 * kp + kt2
                        ksl = slice(128 * kt, 128 * (kt + 1))
                        for hs in range(2):
                            nc.tensor.matmul(
                                s_ps[:, hs, kt2, :, :],
                                kt_sb[h2][64 * hs : 64 * hs + 64, ksl],
                                qg_sb[h2][64 * hs : 64 * hs + 64, :, qsl],
                                start=True, stop=True,
                                tile_position=(64 * hs, 0),
                            )
                    pt = sb3.tile([128, 2, 2, G, QB], BF16, tag="pt")
                    nc.scalar.activation(
                        pt[:], s_ps[:], mybir.ActivationFunctionType.Exp
                    )
                    if kp >= 2 * j:
                        par = kp - 2 * j
                        msk = bm[:, 2 * par : 2 * par + 2, None, :].to_broadcast(
                            (128, 2, G, QB)
                        )
                        for hs in range(2):
                            nc.vector.tensor_mul(pt[:, hs], pt[:, hs], msk)
                    for kt2 in range(2):
                        kt = 2 * kp + kt2
                        for hs in range(2):
                            kv = 2 * h2 + hs
                            nc.tensor.matmul(
                                ctx_ps[h2][:, 256 * hs : 256 * (hs + 1)],
                                v_sb[:, kt, kv, :],
                                pt[:, hs, kt2, :, :],
                                start=(kp == 0 and kt2 == 0 and hs == 0),
                                stop=(kp == nkp - 1 and kt2 == 1),
                                skip_group_check=True,
                            )
            # ---- normalize: ctx / rowsum (row HD of ctx_ps)
            rs = sb.tile([1, 2, 8 * QB], F32R, tag="rs")
            for h2 in range(2):
                nc.vector.tensor_copy(rs[0:1, h2, :], ctx_ps[h2][HD : HD + 1, :])
            hi_st = sb.tile([64, 8, QB], F32R, tag="hist")
            for h2 in range(2):
                bct = pss.tile([128, 1024], F32, tag="scores", name=f"bc{j}{h2}")
                bc = bct[0:HD, 0 : 8 * QB]
                nc.tensor.matmul(
                    bc, ones_sb[:], rs[0:1, h2, :], start=True, stop=True,
                )
                bcs = sb.tile([HD, 8 * QB], F32, tag="bcs")
                nc.vector.reciprocal_approx_fast(out=bcs[:], in_=bc)
                for hs in range(2):
                    ssl = slice(256 * hs, 256 * (hs + 1))
                    if hs == 0:
                        nc.vector.tensor_mul(
                            ctx_sb[0:64, 4 * h2 : 4 * h2 + 4, QB * j : QB * (j + 1)],
                            ctx_ps[h2][0:HD, ssl].rearrange(
                                "p (mq f) -> p mq f", mq=4
                            ),
                            bcs[:, ssl].rearrange("p (mq f) -> p mq f", mq=4),
                        )
                    else:
                        nc.vector.tensor_mul(
                            hi_st[:, 4 * h2 : 4 * h2 + 4, :],
                            ctx_ps[h2][0:HD, ssl].rearrange(
                                "p (mq f) -> p mq f", mq=4
                            ),
                            bcs[:, ssl].rearrange("p (mq f) -> p mq f", mq=4),
                        )
            nc.gpsimd.dma_start(
                ctx_sb[64:128, :, QB * j : QB * (j + 1)], hi_st[:]
            )
            # ---- P3 for t-tile tt once blocks 2tt, 2tt+1 are normalized
            if j % 2 == 1:
                tt = (j - 1) // 2
                for dc in range(2):
                    ps = pss.tile([128, 1024], F32, tag="scores", name=f"pso{tt}{dc}")
                    pso = ps[:, 0:512]
                    for m in range(8):
                        nc.tensor.matmul(
                            pso,
                            ctx_sb[:, m, 128 * tt : 128 * (tt + 1)],
                            wot[:, m, 512 * dc : 512 * (dc + 1)],
                            start=(m == 0), stop=(m == 7),
                        )
                    nc.vector.tensor_copy(
                        out_sb[:, tt, 512 * dc : 512 * (dc + 1)], pso
                    )
                nc.sync.dma_start(
                    out_d.rearrange("(tt p) dcol -> tt p dcol", p=128)[tt],
                    out_sb[:, tt, :],
                )


def _emit_v4(nc, tc, xc_d, xf_d, wq_d, wk_d, wv_d, wo_d, bm_d, on_d, out_d):
    """v4: zero collectives. Every core projects the FULL K^T/V from bf16 x^T,
    one 512-token chunk at a time, interleaved two blocks ahead of the
    attention block that first needs it, so the PE (the bottleneck) never
    stalls and stays at max p-state. P3 runs at the end."""
    from contextlib import ExitStack

    es = ExitStack()
    with es:
        sb = es.enter_context(tc.tile_pool(name="sb", bufs=2))
        sb3 = es.enter_context(tc.tile_pool(name="sb3", bufs=6))
        xfp = es.enter_context(tc.tile_pool(name="xfp", bufs=2))
        res = es.enter_context(tc.tile_pool(name="res", bufs=1))
        pss = es.enter_context(tc.tile_pool(name="pss", bufs=2, space="PSUM"))
        psa = es.enter_context(tc.tile_pool(name="psa", bufs=2, space="PSUM"))
        psc = es.enter_context(tc.tile_pool(name="psc", bufs=2, space="PSUM"))

        # ---------------- resident tensors, in DMA-arrival order of need
        # xc rides the scalar-engine queue, parallel to the sync queue, so
        # Qproj's two inputs stream concurrently from t=0
        xc = res.tile([128, DT, LQ], BF16, tag="xc")           # x^T cyclic cols
        nc.scalar.dma_start(xc[:], xc_d.rearrange("(dt p) q -> p dt q", p=128))
        # Wq sliced per m-tile so Qproj m=0 starts after ~1.25MB of input DMA
        wqt = res.tile([128, DT, HQ * HD], BF16, tag="wqt")
        wqv = wq_d.rearrange("(dt p) h -> p dt h", p=128)
        for m in range(8):
            nc.sync.dma_start(
                wqt[:, :, 128 * m : 128 * (m + 1)], wqv[:, :, 128 * m : 128 * (m + 1)]
            )
        wk = res.tile([128, DT, HKV * HD], BF16, tag="wk")
        nc.sync.dma_start(wk[:], wk_d.rearrange("(dt p) h -> p dt h", p=128))
        wv = res.tile([128, DT, HKV * HD], BF16, tag="wv")
        nc.sync.dma_start(wv[:], wv_d.rearrange("(dt p) h -> p dt h", p=128))
        bm = res.tile([128, 4, QB], BF16, tag="bm")
        nc.sync.dma_start(bm[:], bm_d.rearrange("k p f -> p k f"))
        ones_sb = res.tile([1, HD], F32R, tag="ones")
        nc.sync.dma_start(ones_sb[:], on_d[:])
        # Wo on the gpsimd-triggered queue: parallel to the input stream, so
        # the 4MB f32r load never delays the xf chunk stream on sync
        wot = res.tile([128, 8, D], F32R, tag="wot")
        nc.gpsimd.dma_start(wot[:], wo_d.rearrange("(m p) dcol -> p m dcol", p=128))

        kt_sb = [res.tile([128, T], BF16, tag=f"kt{h2}", name=f"kt{h2}") for h2 in range(2)]
        v_sb = res.tile([128, NKT, HKV, HD + 1], BF16, tag="v")  # [V | 1]
        qg_sb = [
            res.tile([128, G, LQ], BF16, tag=f"qg{h2}", name=f"qg{h2}")
            for h2 in range(2)
        ]
        ctx_sb = res.tile([128, 8, LQ], F32R, tag="ctx")
        out_sb = res.tile([128, 4, D], F32, tag="osb")
        nc.vector.memset(v_sb[:, :, :, HD : HD + 1], 1.0)

        xfv = xf_d.rearrange("(dt p) t -> p dt t", p=128)

        # Fill work: single-PSUM-group closures (KV projection sub-steps, P3
        # column panels) drained one at a time into the PE bubble between each
        # attention group's score matmuls and its ctx matmuls (where the PE
        # otherwise idles waiting for the exp). Keeps the PE saturated and at
        # max p-state. Uses its own 1-bank "aux" tag so the score-tile
        # double-buffer rotation is untouched.
        def kv_sub_items(c, xf):
            items = []
            for h2 in range(2):
                def gok(h2=h2, c=c, xf=xf):
                    ps = psa.tile([128, 512], F32, tag="aux", name=f"ck{c}{h2}")
                    for d in range(DT):
                        nc.tensor.matmul(
                            ps[:], wk[:, d, 128 * h2 : 128 * (h2 + 1)], xf[:, d, :],
                            start=(d == 0), stop=(d == DT - 1),
                        )
                    nc.vector.tensor_copy(kt_sb[h2][:, 512 * c : 512 * (c + 1)], ps[:])
                items.append(gok)
            for tq in range(4):
                def gov(tq=tq, c=c, xf=xf):
                    ps = psa.tile([128, 512], F32, tag="aux", name=f"cv{c}{tq}")
                    psv = ps[:, 0 : HKV * HD]
                    for d in range(DT):
                        nc.tensor.matmul(
                            psv, xf[:, d, 128 * tq : 128 * (tq + 1)], wv[:, d, :],
                            start=(d == 0), stop=(d == DT - 1),
                        )
                    nc.vector.tensor_copy(
                        v_sb[:, 4 * c + tq, :, 0:HD],
                        psv.rearrange("p (h e) -> p h e", h=HKV),
                    )
                items.append(gov)
            return items

        def start_chunk(c):
            xf = xfp.tile([128, DT, 512], BF16, tag="xf", name=f"xf{c}")
            nc.scalar.dma_start(xf[:], xfv[:, :, 512 * c : 512 * (c + 1)])
            return kv_sub_items(c, xf)

        def p3_items(tt):
            items = []
            for dc in range(2):
                def gop(dc=dc, tt=tt):
                    ps = psa.tile([128, 512], F32, tag="aux", name=f"pso{tt}{dc}")
                    for m in range(8):
                        nc.tensor.matmul(
                            ps[:],
                            ctx_sb[:, m, 128 * tt : 128 * (tt + 1)],
                            wot[:, m, 512 * dc : 512 * (dc + 1)],
                            start=(m == 0), stop=(m == 7),
                        )
                    nc.vector.tensor_copy(
                        out_sb[:, tt, 512 * dc : 512 * (dc + 1)], ps[:]
                    )
                items.append(gop)
            def god(tt=tt):
                nc.sync.dma_start(
                    out_d.rearrange("(tt p) dcol -> tt p dcol", p=128)[tt],
                    out_sb[:, tt, :],
                )
            items.append(god)
            return items

        # ---------------- Q^T projection (bf16; HD^-0.5 folded on host)
        for m in range(8):
            ps = psa.tile([128, 512], F32, tag="aux", name=f"psq{m}")
            for d in range(DT):
                nc.tensor.matmul(
                    ps[:], wqt[:, d, 128 * m : 128 * (m + 1)], xc[:, d, :],
                    start=(d == 0), stop=(d == DT - 1),
                )
            nc.vector.tensor_copy(qg_sb[m // 4][:, m % 4, :], ps[:])

        # chunk 0 fully upfront (block 0 reads it immediately)
        for it in start_chunk(0):
            it()
        fill = start_chunk(1)  # chunk 1 drains into block 0's bubbles

        # ---------------- attention blocks with bubble-fill
        for j in range(BPC):
            if 1 <= j and j + 1 < NC:
                fill.extend(start_chunk(j + 1))
            nkp = 2 * (j + 1)
            ctx_ps = [
                psc.tile([HD + 1, 8 * QB], F32, tag="ctx", name=f"ctxps{h2}_{j}")
                for h2 in range(2)
            ]
            rs = sb.tile([1, 2, 8 * QB], F32R, tag="rs")
            hi_st = sb.tile([64, 8, QB], F32R, tag="hist")

            def _norm_h2(jj, h2, rs=rs, hi_st=hi_st, ctx_ps=ctx_ps):
                nc.vector.tensor_copy(rs[0:1, h2, :], ctx_ps[h2][HD : HD + 1, :])
                bct = psa.tile([128, 512], F32, tag="aux", name=f"bc{jj}{h2}")
                bc = bct[0:HD, 0 : 8 * QB]
                nc.tensor.matmul(
                    bc, ones_sb[:], rs[0:1, h2, :], start=True, stop=True,
                )
                bcs = sb.tile([HD, 8 * QB], F32, tag="bcs")
                nc.vector.reciprocal_approx_fast(out=bcs[:], in_=bc)
                for hs in range(2):
                    ssl = slice(256 * hs, 256 * (hs + 1))
                    dst = (
                        ctx_sb[0:64, 4 * h2 : 4 * h2 + 4, QB * jj : QB * (jj + 1)]
                        if hs == 0 else hi_st[:, 4 * h2 : 4 * h2 + 4, :]
                    )
                    nc.vector.tensor_mul(
                        dst,
                        ctx_ps[h2][0:HD, ssl].rearrange("p (mq f) -> p mq f", mq=4),
                        bcs[:, ssl].rearrange("p (mq f) -> p mq f", mq=4),
                    )

            for kp in range(nkp):
                for h2 in range(2):
                    s_t = pss.tile([128, 1024], F32, tag="scores", name="s_t")
                    s_ps = s_t.rearrange(
                        "p (hs kt2 g f) -> p hs kt2 g f", hs=2, kt2=2, g=G
                    )
                    qsl = slice(QB * j, QB * (j + 1))
                    for kt2 in range(2):
                        kt = 2 * kp + kt2
                        ksl = slice(128 * kt, 128 * (kt + 1))
                        for hs in range(2):
                            nc.tensor.matmul(
                                s_ps[:, hs, kt2, :, :],
                                kt_sb[h2][64 * hs : 64 * hs + 64, ksl],
                                qg_sb[h2][64 * hs : 64 * hs + 64, :, qsl],
                                start=True, stop=True,
                                tile_position=(64 * hs, 0),
                            )
                    pt = sb3.tile([128, 2, 2, G, QB], BF16, tag="pt")
                    nc.scalar.activation(
                        pt[:], s_ps[:], mybir.ActivationFunctionType.Exp
                    )
                    if kp >= 2 * j:
                        par = kp - 2 * j
                        msk = bm[:, 2 * par : 2 * par + 2, None, :].to_broadcast(
                            (128, 2, G, QB)
                        )
                        for hs in range(2):
                            nc.vector.tensor_mul(pt[:, hs], pt[:, hs], msk)
                    if fill:
                        fill.pop(0)()
                    for kt2 in range(2):
                        kt = 2 * kp + kt2
                        for hs in range(2):
                            kv = 2 * h2 + hs
                            nc.tensor.matmul(
                                ctx_ps[h2][:, 256 * hs : 256 * (hs + 1)],
                                v_sb[:, kt, kv, :],
                                pt[:, hs, kt2, :, :],
                                start=(kp == 0 and kt2 == 0 and hs == 0),
                                stop=(kp == nkp - 1 and kt2 == 1),
                                skip_group_check=True,
                            )
                    # normalize h2 right after ITS last ctx matmul: starts the
                    # DVE chain a group earlier and frees this ctx_ps bank
                    # sooner (the next block's first ctx matmul waits on it)
                    if kp == nkp - 1:
                        _norm_h2(j, h2)
            nc.gpsimd.dma_start(
                ctx_sb[64:128, :, QB * j : QB * (j + 1)], hi_st[:]
            )
            if j % 2 == 1:
                fill.extend(p3_items((j - 1) // 2))

        # drain whatever fill work remains (P3 of the last t-tile)
        while fill:
            fill.pop(0)()


def _install_ntff_hook():
    """Provide antenv.axon_hooks (absent from this image's antenv) so that
    run_bass_kernel_spmd(trace=True) can NTFF-profile via libaxon_pjrt."""
    import sys as _sys
    import types as _types

    if "antenv.axon_hooks" not in _sys.modules:
        import antenv as _antenv

        mod = _types.ModuleType("antenv.axon_hooks")
        mod._HOOK = None

        def _set(h, _m=mod):
            _m._HOOK = h

        def _get(_m=mod):
            return _m._HOOK

        mod.set_axon_ntff_profile_hook = _set
        mod.get_axon_ntff_profile_hook = _get
        _sys.modules["antenv.axon_hooks"] = mod
        _antenv.axon_hooks = mod
    mod = _sys.modules["antenv.axon_hooks"]
    if mod.get_axon_ntff_profile_hook() is None:
        import trn_agent_boot.trn_boot as _tb

        hook = _tb._ntff_profile_via_ctypes("/opt/axon/libaxon_pjrt.so")
        mod.set_axon_ntff_profile_hook(hook)
    # artifact upload needs a bucket this sandbox doesn't have
    from concourse import bass_utils as _bu

    _bu.upload_artifacts = lambda tmpdir: f"local://{tmpdir}"




def _emit_kv_allgather(nc, tc, res, sb, ps2, xo, wk, wv, kt_sb, v_sb):
    """v2: project K^T/V for OWN tokens only (f32r, from x^T own), AllGather
    the bf16 chunks across the 8 cores, then DMA the gathered (core-major,
    local-order) buffers into the global-order SBUF layouts."""
    from contextlib import ExitStack

    es = ExitStack()
    with es:
        dram = es.enter_context(tc.tile_pool(name="dramkv", bufs=1, space="DRAM"))
        ko_sb = res.tile([128, 2, LQ], BF16, tag="ko_sb")
        vo_sb = res.tile([128, 4, HKV * HD], BF16, tag="vo_sb")
        for h2 in range(2):  # K^T own: [128(2 heads), LQ] per kv-pair
            psk = ps2.tile([128, LQ], F32, tag="scores", name="psk")
            for d in range(DT):
                nc.tensor.matmul(
                    psk[:], wk[:, d, 128 * h2 : 128 * (h2 + 1)], xo[:, d, :],
                    start=(d == 0), stop=(d == DT - 1),
                )
            nc.vector.tensor_copy(ko_sb[:, h2, :], psk[:])
        for tq in range(4):  # V own natural: [128 t, 256]
            psv = ps2.tile([128, HKV * HD], F32, tag="scores", name="psv")
            for d in range(DT):
                nc.tensor.matmul(
                    psv[:], xo[:, d, 128 * tq : 128 * (tq + 1)], wv[:, d, :],
                    start=(d == 0), stop=(d == DT - 1),
                )
            nc.vector.tensor_copy(vo_sb[:, tq, :], psv[:])

        # Two collectives, K first then V: the K-scatter DMAs and the first
        # attention groups overlap the in-flight V gather (a single merged
        # gather measured ~15us slower end-to-end).
        ko_d = dram.tile([2 * 128, LQ], BF16, name="ko_d")
        vo_d = dram.tile([LQ, HKV * HD], BF16, name="vo_d")
        nc.sync.dma_start(ko_d.rearrange("(h2 p) q -> p h2 q", p=128), ko_sb[:])
        nc.sync.dma_start(vo_d.rearrange("(tq p) h -> p tq h", p=128), vo_sb[:])
        kg_d = dram.tile([NC * 2 * 128, LQ], BF16, name="kg_d", addr_space="Shared")
        vg_d = dram.tile([NC * LQ, HKV * HD], BF16, name="vg_d", addr_space="Shared")
        nc.gpsimd.collective_compute(
            "AllGather", mybir.AluOpType.bypass,
            replica_groups=[list(range(NC))],
            ins=[ko_d[:]], outs=[kg_d[:]],
        )
        nc.gpsimd.collective_compute(
            "AllGather", mybir.AluOpType.bypass,
            replica_groups=[list(range(NC))],
            ins=[vo_d[:]], outs=[vg_d[:]],
        )
        # kg rows: 256*c + 128*h2 + 64*hs + d ; cols: 64*j + f (local order)
        kgv = kg_d.rearrange("(c x) (jj f) -> c x jj f", c=NC, f=QB)
        vgv = vg_d.rearrange("(c q) h -> c q h", c=NC)  # [c, 512 local rows, 256]
        for kt in range(NKT):
            c0 = (2 * kt) % NC
            j0 = (2 * kt) // NC
            for h2 in range(2):
                nc.sync.dma_start(
                    kt_sb[h2][:, 128 * kt : 128 * (kt + 1)].rearrange(
                        "p (piece f) -> p piece f", piece=2
                    ),
                    kgv[c0 : c0 + 2, 128 * h2 : 128 * (h2 + 1), j0, :].rearrange(
                        "piece p f -> p piece f"
                    ),
                )
            for pc in range(2):
                nc.sync.dma_start(
                    v_sb[64 * pc : 64 * (pc + 1), kt, :, 0:HD],
                    vgv[c0 + pc, QB * j0 : QB * (j0 + 1), :].rearrange(
                        "p (h e) -> p h e", h=HKV
                    ),
                )


# ---------------------------------------------------------------- host side
_NC_CACHE = None


def _get_nc(mode="v2"):
    global _NC_CACHE
    if _NC_CACHE is None or _NC_CACHE[0] != mode:
        _NC_CACHE = (mode, build_nc(mode))
    return _NC_CACHE[1]


def _prep_in_maps(x, Wq, Wk, Wv, Wo, mode="v2"):
    if mode in ("v3", "v4"):
        return _prep_in_maps_v3(x, Wq, Wk, Wv, Wo, mode)
    xT = np.ascontiguousarray(x[0].T).astype(np.float32)          # [D, T]
    xT_bf = xT.astype(BF16NP) if mode == "v1" else None
    wq_perm = np.empty_like(Wq)
    wo_perm = np.empty_like(Wo)
    for m in range(8):
        wq_perm[:, 128 * m : 128 * m + 64] = Wq[:, 64 * LO[m] : 64 * LO[m] + 64]
        wq_perm[:, 128 * m + 64 : 128 * m + 128] = Wq[:, 64 * HI[m] : 64 * HI[m] + 64]
        wo_perm[128 * m : 128 * m + 64, :] = Wo[64 * LO[m] : 64 * LO[m] + 64, :]
        wo_perm[128 * m + 64 : 128 * m + 128, :] = Wo[64 * HI[m] : 64 * HI[m] + 64, :]
    if mode == "v1":
        wk_n = Wk.astype(BF16NP)
        wv_n = Wv.astype(BF16NP)
    else:
        wk_n = np.ascontiguousarray(Wk, dtype=np.float32)
        wv_n = np.ascontiguousarray(Wv, dtype=np.float32)
    maps = []
    for i in range(NC):
        cols = _local_cols(i)
        m = {
            "xT_own": np.ascontiguousarray(xT[:, cols]),
            "Wq_perm": wq_perm,
            "Wk_n": wk_n,
            "Wv_n": wv_n,
            "Wo_perm": wo_perm,
            "bmask": _band_mask(i),
            "ones_c": np.ones((1, HD), np.float32),
        }
        if mode == "v1":
            m["xT_full"] = xT_bf
        maps.append(m)
    return maps


def _prep_in_maps_v3(x, Wq, Wk, Wv, Wo, mode="v3"):
    xT = np.ascontiguousarray(x[0].T).astype(BF16NP)              # [D, T] bf16
    s = float(HD) ** -0.5
    wq_perm = np.empty_like(Wq)
    wo_perm = np.empty_like(Wo)
    for m in range(8):
        wq_perm[:, 128 * m : 128 * m + 64] = Wq[:, 64 * LO[m] : 64 * LO[m] + 64]
        wq_perm[:, 128 * m + 64 : 128 * m + 128] = Wq[:, 64 * HI[m] : 64 * HI[m] + 64]
        wo_perm[128 * m : 128 * m + 64, :] = Wo[64 * LO[m] : 64 * LO[m] + 64, :]
        wo_perm[128 * m + 64 : 128 * m + 128, :] = Wo[64 * HI[m] : 64 * HI[m] + 64, :]
    wq_bf = (wq_perm * s).astype(BF16NP)
    wo_f32 = np.ascontiguousarray(wo_perm, dtype=np.float32)
    wk_bf = Wk.astype(BF16NP)
    wv_bf = Wv.astype(BF16NP)
    x_head = np.ascontiguousarray(xT[:, 0 : 512 * RC])
    maps = []
    for i in range(NC):
        cols = _local_cols(i)
        m = {
            "xT_cyc": np.ascontiguousarray(xT[:, cols]),
            "Wq_perm": wq_bf,
            "Wk_n": wk_bf,
            "Wv_n": wv_bf,
            "Wo_perm": wo_f32,
            "bmask": _band_mask(i),
            "ones_c": np.ones((1, HD), np.float32),
        }
        if mode == "v3":
            m["xT_ctg"] = np.ascontiguousarray(xT[:, 512 * i : 512 * (i + 1)])
            m["x_head"] = x_head
        else:
            m["xT_full"] = xT
        maps.append(m)
    return maps


def kernel(x, Wq, Wk, Wv, Wo):
    mode = os.environ.get("KERNEL_MODE", "v4")
    nc = _get_nc(mode)
    maps = _prep_in_maps(mode=mode, *(
        np.asarray(x, np.float32),
        np.asarray(Wq, np.float32),
        np.asarray(Wk, np.float32),
        np.asarray(Wv, np.float32),
        np.asarray(Wo, np.float32),
    ))
    trace = bool(int(os.environ.get("KERNEL_TRACE", "0")))
    if trace:
        try:
            _install_ntff_hook()
        except Exception as e:  # profiling is best-effort
            print(f"ntff hook install failed: {e}")
    r = run_bass_kernel_spmd(nc, maps, list(range(NC)), trace=trace)
    out = np.empty((B, T, D), np.float32)
    for i in range(NC):
        out[0, _local_cols(i), :] = r.results[i]["out_loc"]
    if trace:
        kernel.last_exec_time_ns = r.exec_time_ns
        kernel.last_results = r
    return out


if __name__ == "__main__":
    # quick single-core simulation check against a small numpy reference
    pass

